# revision 12
# baseline (speedup 1.0000x reference)
"""CannyNet (blur + sobel + NMS + threshold) on 8 TRN2 NeuronCores.

Strategy
--------
Spatial shard over H: core i handles output rows [512*i, 512*i+512), receiving
a (3, 520, 4104) fp32 slab (8-row H halo, 4-col W pad) sliced on the host.

Device pipeline (per core), all fp32 on the TensorEngine for exactness:
 - stage A: fused vertical-conv + transpose. For each 128-col chunk of the
   image, matmul(stationary = img chunk [rows, 128 wcols],
   moving = band matrix [rows, 244]) computes BOTH 7-tap vertical convs
   (gaussV*sobel-smooth and gaussV*sobel-deriv) transposed into
   [wcols, rows] layout ("L2": W on partitions, H on free dim).
 - stage C: horizontal 7-tap convs as band matmuls (stationary = band
   [128,128] over W, moving = stage-A outputs) -> gx_c, gy_c per channel.
 - pointwise: custom DVE ops (SQSUM) + ACT sqrt -> grad magnitude gm;
   channel sums sgx, sgy; sector masks from |sgy|^2 vs tan^2 * |sgx|^2.
 - NMS: gm shifted +-1 col via exact 0/1 shift matmuls; 8-neighbor maxes
   selected per sector via copy_predicated; final fused threshold+compare.
Output is written W-major ([4096, 512] per core); host transposes back.
"""
import sys

if '/opt/trn_rl_repo' not in sys.path:
    sys.path.insert(0, '/opt/trn_rl_repo')

from contextlib import ExitStack

import numpy as np

import concourse.bass as bass
import concourse.tile as tile
from concourse import bacc, mybir
from concourse.bass_utils import run_bass_kernel_spmd

# ---------------------------------------------------------------- geometry --
H = 4096
W = 4096
C = 3
NCORES = 8
HS = H // NCORES          # 512 output rows per core
HIN = HS + 8              # input rows per core (halo 4+4)
WP = W + 8                # padded width
NPROW = HS + 2            # 514 P/Q/gm rows (out rows -1 .. 513)
# stage-A strips: (row offset in slab, K rows, valid out rows)
STRIPS = [(0, 128, 122), (122, 128, 122), (244, 128, 122), (366, 128, 122),
          (488, 32, 26)]
# W chunks: output col starts (120 wide each, last irregular)
CHUNK_STARTS = [120 * j for j in range(34)] + [W - 120]
NCH = len(CHUNK_STARTS)   # 35
PANEL = 5                 # chunks per DMA panel
THRESH = 10.0
PI_REF = 3.14159

# ------------------------------------------------------- custom DVE ops ----


def _register_custom_ops():
    """Author + register our fused DVE ops into concourse's registry."""
    import concourse.dve_ops as dve_ops
    from concourse.dve_spec import C0, C1, Spec, Src0, Src1, Zero, lower, select, sq
    from concourse.dve_uop import DveOpSpec

    def mk(name, body, reference):
        if name in dve_ops._SUB_OPCODE_FOR_NAME:
            return next(op for op in dve_ops.OPS if op.name == name)
        row = max(dve_ops._SUB_OPCODE_FOR_NAME.values()) + 1
        assert row < 0x20
        dve_ops._SUB_OPCODE_FOR_NAME[name] = row
        spec = Spec(body=body, reference=reference)
        shas = {}
        for ver in ("v3",):
            uops = lower(spec, ver=ver)
            s = DveOpSpec(name=name, opcode=row, uops=uops,
                          rd1_en=True)
            shas[ver] = s.sha(ver)
        op = dve_ops.DveOp(name, spec, subdim=False, uops_sha=shas)
        dve_ops.OPS.append(op)
        dve_ops.CUSTOM_DVE_SPECS[name] = spec
        return op

    # out = in0^2 + in1^2
    sqsum = mk(
        "CANNY_SQSUM",
        sq(Src0) + sq(Src1),
        lambda in0, in1, s0, s1, imm2:
            (in0.astype(np.float32) ** 2 + in1.astype(np.float32) ** 2),
    )
    # out = (in1^2 <= c0 * in0^2) ? c1 : 0     (le-mask on |in1| vs sqrt(c0)|in0|)
    sqle = mk(
        "CANNY_SQLE",
        select(sq(Src0) * C0 < sq(Src1), Zero, C1),
        lambda in0, in1, s0, s1, imm2:
            np.where(in1.astype(np.float32) ** 2 <= s0 * in0.astype(np.float32) ** 2,
                     s1, 0.0).astype(np.float32),
    )
    # out = (in0 * in1 > 0) ? c1 : 0
    sgnpos = mk(
        "CANNY_SGNPOS",
        select(Zero < Src0 * Src1, C1, Zero),
        lambda in0, in1, s0, s1, imm2:
            np.where(in0.astype(np.float32) * in1 > 0, s1, 0.0).astype(np.float32),
    )
    # out = (in0 > in1 and in0 >= c0) ? c1 : 0
    final = mk(
        "CANNY_FINAL",
        select((Src1 < Src0) & (Src0 >= C0), C1, Zero),
        lambda in0, in1, s0, s1, imm2:
            np.where((in0.astype(np.float32) > in1) & (in0 >= s0),
                     s1, 0.0).astype(np.float32),
    )
    return sqsum, sqle, sgnpos, final


# ------------------------------------------------------------ band builders -


def _bands(g):
    """Build all stationary matrices from the 5-tap gaussian g."""
    v1 = np.convolve(g, np.array([1, 2, 1], np.float32)).astype(np.float32)
    v2 = np.convolve(g, np.array([1, 0, -1], np.float32)).astype(np.float32)
    h1 = np.convolve(g, np.array([1, 0, -1], np.float32)).astype(np.float32)
    h2 = np.convolve(g, np.array([1, 2, 1], np.float32)).astype(np.float32)

    def vband(K, nm):
        b = np.zeros((K, 2 * nm), np.float32)
        for m in range(nm):
            for j in range(7):
                b[m + j, m] = v1[j]
                b[m + j, nm + m] = v2[j]
        return b
    vv = vband(128, 122)
    vvp = vband(32, 26)

    def hband(taps):
        b = np.zeros((128, 128), np.float32)
        for wp_ in range(128):
            for j in range(7):
                k = wp_ + j - 3
                if 0 <= k < 128:
                    b[k, wp_] = taps[j]
        return b
    hh1 = hband(h1)
    hh2 = hband(h2)

    return vv, vvp, hh1, hh2


# ------------------------------------------------------------ device build --


def _build_program(reps=1):
    sqsum, sqle, sgnpos, final = _register_custom_ops()
    t2 = float(np.tan(0.5 * PI_REF / 4.0) ** 2)
    T2 = float(np.tan(1.5 * PI_REF / 4.0) ** 2)

    nc = bacc.Bacc("TRN2", target_bir_lowering=False, debug=False,
                   num_devices=NCORES)
    f32 = mybir.dt.float32
    u8 = mybir.dt.uint8

    x_d = nc.dram_tensor("x", [C, HIN, WP], f32, kind="ExternalInput").ap()
    vv_d = nc.dram_tensor("vv", [128, 244], f32, kind="ExternalInput").ap()
    vvp_d = nc.dram_tensor("vvp", [32, 52], f32, kind="ExternalInput").ap()
    hh1_d = nc.dram_tensor("hh1", [128, 128], f32, kind="ExternalInput").ap()
    hh2_d = nc.dram_tensor("hh2", [128, 128], f32, kind="ExternalInput").ap()
    em_d = nc.dram_tensor("em", [128, 2], f32, kind="ExternalInput").ap()
    out_d = nc.dram_tensor("out", [W, HS], f32, kind="ExternalOutput").ap()

    with tile.TileContext(nc) as tc, ExitStack() as ctx:
        consts = ctx.enter_context(tc.tile_pool(name="consts", bufs=1))
        imgp = ctx.enter_context(tc.tile_pool(name="imgp", bufs=2))
        pqp = ctx.enter_context(tc.tile_pool(name="pqp", bufs=2))
        sbp = ctx.enter_context(tc.tile_pool(name="sbp", bufs=2))
        psA = ctx.enter_context(tc.tile_pool(name="psA", bufs=2, space="PSUM"))
        psC = ctx.enter_context(tc.tile_pool(name="psC", bufs=4, space="PSUM"))

        vv_t = consts.tile([128, 244], f32)
        nc.sync.dma_start(vv_t[:], vv_d[:])
        vvp_t = consts.tile([32, 52], f32)
        nc.sync.dma_start(vvp_t[:], vvp_d[:])
        hh1_t = consts.tile([128, 128], f32)
        nc.sync.dma_start(hh1_t[:], hh1_d[:])
        hh2_t = consts.tile([128, 128], f32)
        nc.sync.dma_start(hh2_t[:], hh2_d[:])
        zrow_t = consts.tile([1, NPROW], f32)
        nc.vector.memset(zrow_t[:], 0.0)
        em_t = consts.tile([128, 2], f32)
        nc.sync.dma_start(em_t[:], em_d[:])

        # panel grouping of chunks
        panels = [list(range(i, min(i + PANEL, NCH)))
                  for i in range(0, NCH, PANEL)]

        for panel in [p for _ in range(reps) for p in panels]:
            c0_col = CHUNK_STARTS[panel[0]] - 4 + 4   # xi col of panel start
            c1_col = CHUNK_STARTS[panel[-1]] - 4 + 4 + 128
            ncols = c1_col - c0_col
            # load panel: per (strip, channel) one DMA
            img_ts = {}
            for si, (off, K, nm) in enumerate(STRIPS):
                for ch in range(C):
                    t = imgp.tile([K, ncols], f32, tag=f"img{si}_{ch}")
                    nc.sync.dma_start(
                        t[:], x_d[ch, off:off + K, c0_col:c0_col + ncols])
                    img_ts[(si, ch)] = t

            for j in panel:
                rel = CHUNK_STARTS[j] - 4 + 4 - c0_col  # chunk col within panel
                # ---- stage A: fused vertical conv + transpose ----
                # two 128-row strips share one PSUM bank -> one evac per pair
                pq = {}
                for ch in range(C):
                    pq_t = pqp.tile([128, 2, NPROW], f32, tag=f"pq{ch}")
                    pq[ch] = pq_t
                    for pair in ((0, 1), (2, 3)):
                        pa = psA.tile([128, 2, 2, 122], f32, tag="pa")
                        for k, si in enumerate(pair):
                            off, K, nm = STRIPS[si]
                            nc.tensor.matmul(
                                pa[:, k, :, :],
                                img_ts[(si, ch)][:, rel:rel + 128],
                                vv_t[:], start=True, stop=True)
                        off0 = STRIPS[pair[0]][0]
                        # in dims [strip, pq, row] -> out [pq, strip*122 + row]
                        nc.any.tensor_copy(
                            pq_t[:, :, off0:off0 + 244].rearrange(
                                "p t (s m) -> p s t m", s=2),
                            pa[:])
                    off, K, nm = STRIPS[4]
                    pap = psA.tile([128, 2, 26], f32, tag="pap")
                    nc.tensor.matmul(
                        pap[:], img_ts[(4, ch)][:, rel:rel + 128],
                        vvp_t[:], start=True, stop=True)
                    nc.any.tensor_copy(pq_t[:, :, off:off + nm], pap[:])

                # ---- stage C: horizontal convs ----
                gx_sb, gy_sb = {}, {}
                for name, hh, part, dst in (("gx", hh1_t, 0, gx_sb),
                                            ("gy", hh2_t, 1, gy_sb)):
                    for ch in range(C):
                        t = sbp.tile([128, NPROW], f32, tag=f"{name}{ch}")
                        dst[ch] = t
                        for h0, hn in ((0, 257), (257, 257)):
                            pc = psC.tile([128, 257], f32, tag="pc")
                            nc.tensor.matmul(
                                pc[:, 0:hn],
                                hh[:],
                                pq[ch][:, part, h0:h0 + hn],
                                start=True, stop=True)
                            nc.any.tensor_copy(t[:, h0:h0 + hn], pc[:, 0:hn])

                # ---- pointwise: magnitude + channel sums ----
                mag = {}
                for ch in range(C):
                    ss = sbp.tile([128, NPROW], f32, tag=f"ss{ch}")
                    nc.vector._custom_dve(sqsum, out=ss[:], in0=gx_sb[ch][:],
                                          in1=gy_sb[ch][:])
                    m = sbp.tile([128, NPROW], f32, tag=f"mag{ch}")
                    nc.scalar.sqrt(m[:], ss[:])
                    mag[ch] = m
                gm = sbp.tile([128, NPROW], f32, tag="gm")
                nc.vector.tensor_add(gm[:], mag[0][:], mag[1][:])
                nc.vector.tensor_add(gm[:], gm[:], mag[2][:])
                sgx = sbp.tile([128, NPROW], f32, tag="sgx")
                nc.vector.tensor_add(sgx[:], gx_sb[0][:], gx_sb[1][:])
                nc.vector.tensor_add(sgx[:], sgx[:], gx_sb[2][:])
                sgy = sbp.tile([128, NPROW], f32, tag="sgy")
                nc.vector.tensor_add(sgy[:], gy_sb[0][:], gy_sb[1][:])
                nc.vector.tensor_add(sgy[:], sgy[:], gy_sb[2][:])

                # global top/bottom edge: zero the out-of-image gm halo rows
                nc.scalar.activation(gm[:, 0:1], gm[:, 0:1],
                                     mybir.ActivationFunctionType.Copy,
                                     scale=em_t[:, 0:1])
                nc.scalar.activation(gm[:, NPROW - 1:NPROW],
                                     gm[:, NPROW - 1:NPROW],
                                     mybir.ActivationFunctionType.Copy,
                                     scale=em_t[:, 1:2])

                # ---- sector masks ----
                let2 = sbp.tile([128, NPROW], u8, tag="let2")
                nc.vector._custom_dve(sqle, out=let2[:], in0=sgx[:],
                                      in1=sgy[:], s0=t2, s1=1.0)
                leT2 = sbp.tile([128, NPROW], u8, tag="leT2")
                nc.vector._custom_dve(sqle, out=leT2[:], in0=sgx[:],
                                      in1=sgy[:], s0=T2, s1=1.0)
                sgp = sbp.tile([128, NPROW], u8, tag="sgp")
                nc.vector._custom_dve(sgnpos, out=sgp[:], in0=sgx[:],
                                      in1=sgy[:], s0=0.0, s1=1.0)

                # ---- NMS shifts via partition-offset SBUF->SBUF DMA ----
                gmL = sbp.tile([128, NPROW], f32, tag="gmL")
                gmR = sbp.tile([128, NPROW], f32, tag="gmR")
                nc.sync.dma_start(gmL[1:128, :], gm[0:127, :])
                nc.sync.dma_start(gmR[0:127, :], gm[1:128, :])
                if j == 0:
                    # left neighbor of global col 0 is zero padding
                    nc.sync.dma_start(gmL[4:5, :], zrow_t[:])
                if j == NCH - 1:
                    # right neighbor of global col 4095 is zero padding
                    nc.sync.dma_start(gmR[123:124, :], zrow_t[:])

                # ---- NMS: sector-selected neighbor max, compare, threshold --
                # window rows p in [1, 513) = the 512 output rows
                M1 = sbp.tile([128, HS], f32, tag="M1")
                nc.vector.tensor_tensor(out=M1[:], in0=gmR[:, 2:2 + HS],
                                        in1=gmL[:, 0:HS],
                                        op=mybir.AluOpType.max)
                ND = sbp.tile([128, HS], f32, tag="ND")
                nc.vector.tensor_tensor(out=ND[:], in0=gmL[:, 2:2 + HS],
                                        in1=gmR[:, 0:HS],
                                        op=mybir.AluOpType.max)
                nc.vector.copy_predicated(ND[:], sgp[:, 1:1 + HS], M1[:])
                NMAX = sbp.tile([128, HS], f32, tag="NMAX")
                nc.vector.tensor_tensor(out=NMAX[:], in0=gm[:, 2:2 + HS],
                                        in1=gm[:, 0:HS],
                                        op=mybir.AluOpType.max)
                nc.vector.copy_predicated(NMAX[:], leT2[:, 1:1 + HS], ND[:])
                M0 = sbp.tile([128, HS], f32, tag="M0")
                nc.vector.tensor_tensor(out=M0[:], in0=gmR[:, 1:1 + HS],
                                        in1=gmL[:, 1:1 + HS],
                                        op=mybir.AluOpType.max)
                nc.vector.copy_predicated(NMAX[:], let2[:, 1:1 + HS], M0[:])

                out_t = sbp.tile([128, HS], f32, tag="out")
                nc.vector._custom_dve(final, out=out_t[:],
                                      in0=gm[:, 1:1 + HS], in1=NMAX[:],
                                      s0=THRESH, s1=1.0)

                # ---- store: partitions [4:124) -> out rows [o_j, o_j+120) --
                oj = CHUNK_STARTS[j]
                nc.sync.dma_start(out_d[oj:oj + 120, :], out_t[4:124, :])

    nc.compile()
    return nc


_PROGRAM = None
PROFILE = False
LAST_EXEC_NS = None


def _get_program():
    global _PROGRAM
    if _PROGRAM is None:
        _PROGRAM = _build_program()
    return _PROGRAM


# ------------------------------------------------------------------ kernel --


def _make_in_maps(inputs):
    img = np.asarray(inputs["img"], dtype=np.float32)
    g = np.asarray(inputs["gauss_h"], dtype=np.float32).reshape(-1)
    assert img.shape == (1, C, H, W) and g.shape == (5,)

    vv, vvp, hh1, hh2 = _bands(g)

    padded = np.zeros((C, H + 8, WP), np.float32)
    padded[:, 4:4 + H, 4:4 + W] = img[0]

    in_maps = []
    for i in range(NCORES):
        em = np.ones((128, 2), np.float32)
        if i == 0:
            em[:, 0] = 0.0
        if i == NCORES - 1:
            em[:, 1] = 0.0
        xi = np.ascontiguousarray(padded[:, HS * i: HS * i + HIN, :])
        in_maps.append({
            "x": xi, "vv": vv, "vvp": vvp, "hh1": hh1, "hh2": hh2, "em": em,
        })
    return in_maps


def kernel(img, gauss_h, gauss_v, sobel_h, sobel_v, dir_w):
    in_maps = _make_in_maps({"img": img, "gauss_h": gauss_h})
    nc = _get_program()

    global LAST_EXEC_NS
    res = run_bass_kernel_spmd(nc, in_maps, list(range(NCORES)),
                               trace=bool(PROFILE))
    LAST_EXEC_NS = res.exec_time_ns
    out = np.empty((H, W), np.float32)
    for i in range(NCORES):
        out[HS * i: HS * (i + 1), :] = res.results[i]["out"].T
    return out.reshape(1, 1, H, W)


# revision 15
# speedup vs baseline: 1.0243x; 1.0243x over previous
"""CannyNet (blur + sobel + NMS + threshold) on 8 TRN2 NeuronCores.

Strategy
--------
Spatial shard over H: core i handles output rows [512*i, 512*i+512), receiving
a (3, 520, 4104) fp32 slab (8-row H halo, 4-col W pad) sliced on the host.

Device pipeline (per core), all fp32 on the TensorEngine for exactness:
 - stage A: fused vertical-conv + transpose. For each 128-col chunk of the
   image, matmul(stationary = img chunk [rows, 128 wcols],
   moving = band matrix [rows, 244]) computes BOTH 7-tap vertical convs
   (gaussV*sobel-smooth and gaussV*sobel-deriv) transposed into
   [wcols, rows] layout ("L2": W on partitions, H on free dim).
 - stage C: horizontal 7-tap convs as band matmuls (stationary = band
   [128,128] over W, moving = stage-A outputs) -> gx_c, gy_c per channel.
 - pointwise: custom DVE ops (SQSUM) + ACT sqrt -> grad magnitude gm;
   channel sums sgx, sgy; sector masks from |sgy|^2 vs tan^2 * |sgx|^2.
 - NMS: gm shifted +-1 col via exact 0/1 shift matmuls; 8-neighbor maxes
   selected per sector via copy_predicated; final fused threshold+compare.
Output is written W-major ([4096, 512] per core); host transposes back.
"""
import sys

if '/opt/trn_rl_repo' not in sys.path:
    sys.path.insert(0, '/opt/trn_rl_repo')

from contextlib import ExitStack

import numpy as np

import concourse.bass as bass
import concourse.tile as tile
from concourse import bacc, mybir
from concourse.bass_utils import run_bass_kernel_spmd

# ---------------------------------------------------------------- geometry --
H = 4096
W = 4096
C = 3
NCORES = 8
HS = H // NCORES          # 512 output rows per core
HIN = HS + 8              # input rows per core (halo 4+4)
WP = W + 8                # padded width
NPROW = HS + 2            # 514 P/Q/gm rows (out rows -1 .. 513)
# stage-A strips: (row offset in slab, K rows, valid out rows)
STRIPS = [(0, 128, 122), (122, 128, 122), (244, 128, 122), (366, 128, 122),
          (488, 32, 26)]
# W chunks: output col starts (120 wide each, last irregular)
CHUNK_STARTS = [120 * j for j in range(34)] + [W - 120]
NCH = len(CHUNK_STARTS)   # 35
PANEL = 5                 # chunks per DMA panel
THRESH = 10.0
PI_REF = 3.14159

# ------------------------------------------------------- custom DVE ops ----


def _register_custom_ops():
    """Author + register our fused DVE ops into concourse's registry."""
    import concourse.dve_ops as dve_ops
    from concourse.dve_spec import C0, C1, Spec, Src0, Src1, Zero, lower, select, sq
    from concourse.dve_uop import DveOpSpec

    def mk(name, body, reference):
        if name in dve_ops._SUB_OPCODE_FOR_NAME:
            return next(op for op in dve_ops.OPS if op.name == name)
        row = max(dve_ops._SUB_OPCODE_FOR_NAME.values()) + 1
        assert row < 0x20
        dve_ops._SUB_OPCODE_FOR_NAME[name] = row
        spec = Spec(body=body, reference=reference)
        shas = {}
        for ver in ("v3",):
            uops = lower(spec, ver=ver)
            s = DveOpSpec(name=name, opcode=row, uops=uops,
                          rd1_en=True)
            shas[ver] = s.sha(ver)
        op = dve_ops.DveOp(name, spec, subdim=False, uops_sha=shas)
        dve_ops.OPS.append(op)
        dve_ops.CUSTOM_DVE_SPECS[name] = spec
        return op

    # out = in0^2 + in1^2
    sqsum = mk(
        "CANNY_SQSUM",
        sq(Src0) + sq(Src1),
        lambda in0, in1, s0, s1, imm2:
            (in0.astype(np.float32) ** 2 + in1.astype(np.float32) ** 2),
    )
    # out = (in1^2 <= c0 * in0^2) ? c1 : 0     (le-mask on |in1| vs sqrt(c0)|in0|)
    sqle = mk(
        "CANNY_SQLE",
        select(sq(Src0) * C0 < sq(Src1), Zero, C1),
        lambda in0, in1, s0, s1, imm2:
            np.where(in1.astype(np.float32) ** 2 <= s0 * in0.astype(np.float32) ** 2,
                     s1, 0.0).astype(np.float32),
    )
    # out = (in0 * in1 > 0) ? c1 : 0
    sgnpos = mk(
        "CANNY_SGNPOS",
        select(Zero < Src0 * Src1, C1, Zero),
        lambda in0, in1, s0, s1, imm2:
            np.where(in0.astype(np.float32) * in1 > 0, s1, 0.0).astype(np.float32),
    )
    # out = (in0 > in1 and in0 >= c0) ? c1 : 0
    final = mk(
        "CANNY_FINAL",
        select((Src1 < Src0) & (Src0 >= C0), C1, Zero),
        lambda in0, in1, s0, s1, imm2:
            np.where((in0.astype(np.float32) > in1) & (in0 >= s0),
                     s1, 0.0).astype(np.float32),
    )
    return sqsum, sqle, sgnpos, final


# ------------------------------------------------------------ band builders -


def _bands(g):
    """Build all stationary matrices from the 5-tap gaussian g."""
    v1 = np.convolve(g, np.array([1, 2, 1], np.float32)).astype(np.float32)
    v2 = np.convolve(g, np.array([1, 0, -1], np.float32)).astype(np.float32)
    h1 = np.convolve(g, np.array([1, 0, -1], np.float32)).astype(np.float32)
    h2 = np.convolve(g, np.array([1, 2, 1], np.float32)).astype(np.float32)

    def vband(K, nm):
        b = np.zeros((K, 2 * nm), np.float32)
        for m in range(nm):
            for j in range(7):
                b[m + j, m] = v1[j]
                b[m + j, nm + m] = v2[j]
        return b
    vv = vband(128, 122)
    vvp = vband(32, 26)

    def hband(taps):
        b = np.zeros((128, 128), np.float32)
        for wp_ in range(128):
            for j in range(7):
                k = wp_ + j - 3
                if 0 <= k < 128:
                    b[k, wp_] = taps[j]
        return b
    hh1 = hband(h1)
    hh2 = hband(h2)

    sl = np.zeros((128, 128), np.float32)
    sr = np.zeros((128, 128), np.float32)
    for wp_ in range(128):
        if wp_ - 1 >= 0:
            sl[wp_ - 1, wp_] = 1.0
        if wp_ + 1 < 128:
            sr[wp_ + 1, wp_] = 1.0
    sl0 = sl.copy()
    sl0[3, 4] = 0.0          # chunk 0: left edge zero-pad
    srl = sr.copy()
    srl[124, 123] = 0.0      # last chunk: right edge zero-pad
    return vv, vvp, hh1, hh2, sl, sr, sl0, srl


# ------------------------------------------------------------ device build --


def _build_program(reps=1, shift_mode="pe", a_shared_w=False):
    sqsum, sqle, sgnpos, final = _register_custom_ops()
    t2 = float(np.tan(0.5 * PI_REF / 4.0) ** 2)
    T2 = float(np.tan(1.5 * PI_REF / 4.0) ** 2)

    nc = bacc.Bacc("TRN2", target_bir_lowering=False, debug=False,
                   num_devices=NCORES)
    f32 = mybir.dt.float32
    u8 = mybir.dt.uint8

    x_d = nc.dram_tensor("x", [C, HIN, WP], f32, kind="ExternalInput").ap()
    vv_d = nc.dram_tensor("vv", [128, 244], f32, kind="ExternalInput").ap()
    vvp_d = nc.dram_tensor("vvp", [32, 52], f32, kind="ExternalInput").ap()
    hh1_d = nc.dram_tensor("hh1", [128, 128], f32, kind="ExternalInput").ap()
    hh2_d = nc.dram_tensor("hh2", [128, 128], f32, kind="ExternalInput").ap()
    em_d = nc.dram_tensor("em", [128, 2], f32, kind="ExternalInput").ap()
    sl_d = nc.dram_tensor("sl", [128, 128], f32, kind="ExternalInput").ap()
    sr_d = nc.dram_tensor("sr", [128, 128], f32, kind="ExternalInput").ap()
    sl0_d = nc.dram_tensor("sl0", [128, 128], f32, kind="ExternalInput").ap()
    srl_d = nc.dram_tensor("srl", [128, 128], f32, kind="ExternalInput").ap()
    out_d = nc.dram_tensor("out", [W, HS], f32, kind="ExternalOutput").ap()

    with tile.TileContext(nc) as tc, ExitStack() as ctx:
        consts = ctx.enter_context(tc.tile_pool(name="consts", bufs=1))
        imgp = ctx.enter_context(tc.tile_pool(name="imgp", bufs=2))
        pqp = ctx.enter_context(tc.tile_pool(name="pqp", bufs=2))
        sbp = ctx.enter_context(tc.tile_pool(name="sbp", bufs=2))
        psA = ctx.enter_context(tc.tile_pool(name="psA", bufs=2, space="PSUM"))
        psC = ctx.enter_context(tc.tile_pool(name="psC", bufs=4, space="PSUM"))

        vv_t = consts.tile([128, 244], f32)
        nc.sync.dma_start(vv_t[:], vv_d[:])
        vvp_t = consts.tile([32, 52], f32)
        nc.sync.dma_start(vvp_t[:], vvp_d[:])
        hh1_t = consts.tile([128, 128], f32)
        nc.sync.dma_start(hh1_t[:], hh1_d[:])
        hh2_t = consts.tile([128, 128], f32)
        nc.sync.dma_start(hh2_t[:], hh2_d[:])
        zrow_t = consts.tile([1, NPROW], f32)
        nc.vector.memset(zrow_t[:], 0.0)
        sl_t = consts.tile([128, 128], f32)
        nc.sync.dma_start(sl_t[:], sl_d[:])
        sr_t = consts.tile([128, 128], f32)
        nc.sync.dma_start(sr_t[:], sr_d[:])
        sl0_t = consts.tile([128, 128], f32)
        nc.sync.dma_start(sl0_t[:], sl0_d[:])
        srl_t = consts.tile([128, 128], f32)
        nc.sync.dma_start(srl_t[:], srl_d[:])
        em_t = consts.tile([128, 2], f32)
        nc.sync.dma_start(em_t[:], em_d[:])

        # panel grouping of chunks
        panels = [list(range(i, min(i + PANEL, NCH)))
                  for i in range(0, NCH, PANEL)]

        for panel in [p for _ in range(reps) for p in panels]:
            c0_col = CHUNK_STARTS[panel[0]] - 4 + 4   # xi col of panel start
            c1_col = CHUNK_STARTS[panel[-1]] - 4 + 4 + 128
            ncols = c1_col - c0_col
            # load panel: per (strip, channel) one DMA
            img_ts = {}
            for si, (off, K, nm) in enumerate(STRIPS):
                for ch in range(C):
                    t = imgp.tile([K, ncols], f32, tag=f"img{si}_{ch}")
                    nc.sync.dma_start(
                        t[:], x_d[ch, off:off + K, c0_col:c0_col + ncols])
                    img_ts[(si, ch)] = t

            for j in panel:
                rel = CHUNK_STARTS[j] - 4 + 4 - c0_col  # chunk col within panel
                # ---- stage A: fused vertical conv + transpose ----
                # two 128-row strips share one PSUM bank -> one evac per pair
                pq = {}
                for ch in range(C):
                    pq_t = pqp.tile([128, 2, NPROW], f32, tag=f"pq{ch}")
                    pq[ch] = pq_t
                    for pair in ((0, 1), (2, 3)):
                        pa = psA.tile([128, 2, 2, 122], f32, tag="pa")
                        for k, si in enumerate(pair):
                            off, K, nm = STRIPS[si]
                            lhs = (img_ts[(0, 0)][:, 0:128] if a_shared_w
                                   else img_ts[(si, ch)][:, rel:rel + 128])
                            nc.tensor.matmul(
                                pa[:, k, :, :], lhs,
                                vv_t[:], start=True, stop=True)
                        off0 = STRIPS[pair[0]][0]
                        # in dims [strip, pq, row] -> out [pq, strip*122 + row]
                        nc.any.tensor_copy(
                            pq_t[:, :, off0:off0 + 244].rearrange(
                                "p t (s m) -> p s t m", s=2),
                            pa[:])
                    off, K, nm = STRIPS[4]
                    pap = psA.tile([128, 2, 26], f32, tag="pap")
                    nc.tensor.matmul(
                        pap[:], img_ts[(4, ch)][:, rel:rel + 128],
                        vvp_t[:], start=True, stop=True)
                    nc.any.tensor_copy(pq_t[:, :, off:off + nm], pap[:])

                # ---- stage C: horizontal convs ----
                gx_sb, gy_sb = {}, {}
                for name, hh, part, dst in (("gx", hh1_t, 0, gx_sb),
                                            ("gy", hh2_t, 1, gy_sb)):
                    for ch in range(C):
                        t = sbp.tile([128, NPROW], f32, tag=f"{name}{ch}")
                        dst[ch] = t
                        for h0, hn in ((0, 257), (257, 257)):
                            pc = psC.tile([128, 257], f32, tag="pc")
                            nc.tensor.matmul(
                                pc[:, 0:hn],
                                hh[:],
                                pq[ch][:, part, h0:h0 + hn],
                                start=True, stop=True)
                            nc.any.tensor_copy(t[:, h0:h0 + hn], pc[:, 0:hn])

                # ---- pointwise: magnitude + channel sums ----
                mag = {}
                for ch in range(C):
                    ss = sbp.tile([128, NPROW], f32, tag=f"ss{ch}")
                    nc.vector._custom_dve(sqsum, out=ss[:], in0=gx_sb[ch][:],
                                          in1=gy_sb[ch][:])
                    m = sbp.tile([128, NPROW], f32, tag=f"mag{ch}")
                    nc.scalar.sqrt(m[:], ss[:])
                    mag[ch] = m
                gm = sbp.tile([128, NPROW], f32, tag="gm")
                nc.vector.tensor_add(gm[:], mag[0][:], mag[1][:])
                nc.vector.tensor_add(gm[:], gm[:], mag[2][:])
                sgx = sbp.tile([128, NPROW], f32, tag="sgx")
                nc.vector.tensor_add(sgx[:], gx_sb[0][:], gx_sb[1][:])
                nc.vector.tensor_add(sgx[:], sgx[:], gx_sb[2][:])
                sgy = sbp.tile([128, NPROW], f32, tag="sgy")
                nc.vector.tensor_add(sgy[:], gy_sb[0][:], gy_sb[1][:])
                nc.vector.tensor_add(sgy[:], sgy[:], gy_sb[2][:])

                # global top/bottom edge: zero the out-of-image gm halo rows
                nc.vector.tensor_tensor(out=gm[:, 0:1], in0=gm[:, 0:1],
                                        in1=em_t[:, 0:1],
                                        op=mybir.AluOpType.mult)
                nc.vector.tensor_tensor(out=gm[:, NPROW - 1:NPROW],
                                        in0=gm[:, NPROW - 1:NPROW],
                                        in1=em_t[:, 1:2],
                                        op=mybir.AluOpType.mult)

                # ---- sector masks ----
                let2 = sbp.tile([128, NPROW], u8, tag="let2")
                nc.vector._custom_dve(sqle, out=let2[:], in0=sgx[:],
                                      in1=sgy[:], s0=t2, s1=1.0)
                leT2 = sbp.tile([128, NPROW], u8, tag="leT2")
                nc.vector._custom_dve(sqle, out=leT2[:], in0=sgx[:],
                                      in1=sgy[:], s0=T2, s1=1.0)
                sgp = sbp.tile([128, NPROW], u8, tag="sgp")
                nc.vector._custom_dve(sgnpos, out=sgp[:], in0=sgx[:],
                                      in1=sgy[:], s0=0.0, s1=1.0)

                # ---- NMS shifts: gmL[w] = gm[w-1], gmR[w] = gm[w+1] ----
                gmL = sbp.tile([128, NPROW], f32, tag="gmL")
                gmR = sbp.tile([128, NPROW], f32, tag="gmR")
                if shift_mode == "dma":
                    nc.sync.dma_start(gmL[1:128, :], gm[0:127, :])
                    nc.sync.dma_start(gmR[0:127, :], gm[1:128, :])
                    if j == 0:
                        nc.sync.dma_start(gmL[4:5, :], zrow_t[:])
                    if j == NCH - 1:
                        nc.sync.dma_start(gmR[123:124, :], zrow_t[:])
                else:
                    slb = sl0_t if j == 0 else sl_t
                    srb = srl_t if j == NCH - 1 else sr_t
                    for t, b in ((gmL, slb), (gmR, srb)):
                        for h0, hn in ((0, 257), (257, 257)):
                            pc = psC.tile([128, 257], f32, tag="pc")
                            nc.tensor.matmul(pc[:, 0:hn], b[:],
                                             gm[:, h0:h0 + hn],
                                             start=True, stop=True)
                            nc.any.tensor_copy(t[:, h0:h0 + hn], pc[:, 0:hn])

                # ---- NMS: sector-selected neighbor max, compare, threshold --
                # window rows p in [1, 513) = the 512 output rows
                M1 = sbp.tile([128, HS], f32, tag="M1")
                nc.vector.tensor_tensor(out=M1[:], in0=gmR[:, 2:2 + HS],
                                        in1=gmL[:, 0:HS],
                                        op=mybir.AluOpType.max)
                ND = sbp.tile([128, HS], f32, tag="ND")
                nc.vector.tensor_tensor(out=ND[:], in0=gmL[:, 2:2 + HS],
                                        in1=gmR[:, 0:HS],
                                        op=mybir.AluOpType.max)
                nc.vector.copy_predicated(ND[:], sgp[:, 1:1 + HS], M1[:])
                NMAX = sbp.tile([128, HS], f32, tag="NMAX")
                nc.vector.tensor_tensor(out=NMAX[:], in0=gm[:, 2:2 + HS],
                                        in1=gm[:, 0:HS],
                                        op=mybir.AluOpType.max)
                nc.vector.copy_predicated(NMAX[:], leT2[:, 1:1 + HS], ND[:])
                M0 = sbp.tile([128, HS], f32, tag="M0")
                nc.vector.tensor_tensor(out=M0[:], in0=gmR[:, 1:1 + HS],
                                        in1=gmL[:, 1:1 + HS],
                                        op=mybir.AluOpType.max)
                nc.vector.copy_predicated(NMAX[:], let2[:, 1:1 + HS], M0[:])

                out_t = sbp.tile([128, HS], f32, tag="out")
                nc.vector._custom_dve(final, out=out_t[:],
                                      in0=gm[:, 1:1 + HS], in1=NMAX[:],
                                      s0=THRESH, s1=1.0)

                # ---- store: partitions [4:124) -> out rows [o_j, o_j+120) --
                oj = CHUNK_STARTS[j]
                nc.sync.dma_start(out_d[oj:oj + 120, :], out_t[4:124, :])

    nc.compile()
    return nc


_PROGRAM = None
PROFILE = False
LAST_EXEC_NS = None


def _get_program():
    global _PROGRAM
    if _PROGRAM is None:
        _PROGRAM = _build_program()
    return _PROGRAM


# ------------------------------------------------------------------ kernel --


def _make_in_maps(inputs):
    img = np.asarray(inputs["img"], dtype=np.float32)
    g = np.asarray(inputs["gauss_h"], dtype=np.float32).reshape(-1)
    assert img.shape == (1, C, H, W) and g.shape == (5,)

    vv, vvp, hh1, hh2, sl, sr, sl0, srl = _bands(g)

    padded = np.zeros((C, H + 8, WP), np.float32)
    padded[:, 4:4 + H, 4:4 + W] = img[0]

    in_maps = []
    for i in range(NCORES):
        em = np.ones((128, 2), np.float32)
        if i == 0:
            em[:, 0] = 0.0
        if i == NCORES - 1:
            em[:, 1] = 0.0
        xi = np.ascontiguousarray(padded[:, HS * i: HS * i + HIN, :])
        in_maps.append({
            "x": xi, "vv": vv, "vvp": vvp, "hh1": hh1, "hh2": hh2, "em": em,
            "sl": sl, "sr": sr, "sl0": sl0, "srl": srl,
        })
    return in_maps


def kernel(img, gauss_h, gauss_v, sobel_h, sobel_v, dir_w):
    in_maps = _make_in_maps({"img": img, "gauss_h": gauss_h})
    nc = _get_program()

    global LAST_EXEC_NS
    res = run_bass_kernel_spmd(nc, in_maps, list(range(NCORES)),
                               trace=bool(PROFILE))
    LAST_EXEC_NS = res.exec_time_ns
    out = np.empty((H, W), np.float32)
    for i in range(NCORES):
        out[HS * i: HS * (i + 1), :] = res.results[i]["out"].T
    return out.reshape(1, 1, H, W)
